# revision 10
# baseline (speedup 1.0000x reference)
"""Distributed Trainium2 Bass kernel for nn_Attention_72791105732731.

Reference computation (S=16384, H=4096):
    score_ = hidden @ W1.T            # [S,H]
    h_t    = hidden[-1]
    score  = score_ @ h_t             # [S]
    aw     = softmax(score)
    ctx    = hidden.T @ aw            # [H]
    av     = tanh(W2 @ concat(ctx, h_t))
    return (av, aw)

Key algebraic identity: score = hidden @ (W1.T @ h_t) — reassociation turns the
550-GFLOP fc1 matmul into two matvecs, making the problem memory-bound.

Distribution over 8 cores:
  - hidden sharded over S (2048 rows/core), host-pre-transposed to [H, 2048]
    so the score contraction (over H) sits on the partition axis for TensorE.
  - W1 sharded over rows (512/core): partial v = W1_shard.T @ h_t_shard,
    AllReduce(v).
  - softmax via block-local (max, sumexp) stats + AllGather of per-core stats.
  - context partial per core via DVE fused multiply-reduce, AllReduce(ctx).
  - W2 sharded over output rows (512/core), host-pre-transposed; fc2 computed
    per-shard, host gathers.

Compute dtype bf16 (validated offline: aw absmax err ~7e-5, av ~9e-3 vs fp32
reference — well within the 2e-2 gate); all accumulations fp32 (PSUM / DVE
accum / stats math).
"""

from contextlib import ExitStack

import ml_dtypes
import numpy as np

import concourse.bass as bass
import concourse.tile as tile
from concourse import bacc, mybir
from concourse.bass_utils import run_bass_kernel_spmd

F32 = mybir.dt.float32
BF16 = mybir.dt.bfloat16
AF = mybir.ActivationFunctionType
ALU = mybir.AluOpType

N_CORES = 8
S = 16384
H = 4096


def build_graph(n_cores=N_CORES, s_shard=S // N_CORES, h=H, blk=512,
                m_shard=H // N_CORES):
    """Build the SPMD single-core Bass graph (identical on every core)."""
    nb = s_shard // blk          # score/softmax blocks per core
    ht_tiles = h // 128          # h-tiles (partition tiles) = 32
    pm_cols = h // 128           # columns of the partition-major h_t layout
    jt = (h // n_cores) // 128   # W1 row tiles per core = 4
    k2 = 2 * h // 128            # fc2 contraction tiles = 64
    sub = 4                      # hidden sub-DMAs per block
    assert ht_tiles % sub == 0

    nc = bacc.Bacc("TRN2", target_bir_lowering=False, debug=False,
                   num_devices=n_cores)

    # ---- I/O ----
    hid_t = nc.dram_tensor("hid_t", [h, s_shard], BF16, kind="ExternalInput")
    w1s = nc.dram_tensor("w1s", [h // n_cores, h], BF16, kind="ExternalInput")
    w2t = nc.dram_tensor("w2t", [2 * h, m_shard], BF16, kind="ExternalInput")
    ht_pm = nc.dram_tensor("ht_pm", [128, pm_cols], BF16, kind="ExternalInput")
    ht_loc = nc.dram_tensor("ht_loc", [128, jt], BF16, kind="ExternalInput")
    out_w = nc.dram_tensor("out_w", [s_shard], F32, kind="ExternalOutput")
    out_av = nc.dram_tensor("out_av", [m_shard], F32, kind="ExternalOutput")

    groups = [list(range(n_cores))]

    with tile.TileContext(nc) as tc, ExitStack() as ctx:
        dram = ctx.enter_context(tc.tile_pool(name="dram", bufs=1, space="DRAM"))
        psum = ctx.enter_context(tc.tile_pool(name="psum", bufs=1, space="PSUM"))
        psum2 = ctx.enter_context(tc.tile_pool(name="psum2", bufs=2, space="PSUM"))
        sb = ctx.enter_context(tc.tile_pool(name="sb", bufs=1))
        sb2 = ctx.enter_context(tc.tile_pool(name="sb2", bufs=2))
        hidp = ctx.enter_context(tc.tile_pool(name="hidp", bufs=2))
        w1p = ctx.enter_context(tc.tile_pool(name="w1p", bufs=1))

        ones = sb.tile([1, 128], F32, name="ones")
        nc.vector.memset(ones[:], 1.0)

        ht_loc_sb = sb.tile([128, jt], BF16, name="ht_loc_sb")
        nc.sync.dma_start(ht_loc_sb[:], ht_loc.ap())

        # ---- v = W1.T @ h_t (partial over this core's W1 rows) ----
        # PSUM start=True zeroes a whole 2KB bank region, so each column's
        # accumulation group must finish (and be copied out) before the next
        # column's start re-zeroes the bank: iterate columns outer, j inner.
        w1_sbs = []
        for j in range(jt):
            w1_sb = w1p.tile([128, h], BF16, name=f"w1_sb{j}", tag=f"w1{j}")
            nc.sync.dma_start(w1_sb[:], w1s.ap()[j * 128:(j + 1) * 128, :])
            w1_sbs.append(w1_sb)
        v_ps = psum.tile([128, pm_cols], F32, name="v_ps")
        v_sb = sb.tile([128, pm_cols], F32, name="v_sb")
        for i in range(ht_tiles):
            for j in range(jt):
                nc.tensor.matmul(
                    v_ps[:, i:i + 1],
                    lhsT=w1_sbs[j][:, i * 128:(i + 1) * 128],
                    rhs=ht_loc_sb[:, j:j + 1],
                    start=(j == 0), stop=(j == jt - 1),
                )
            nc.scalar.copy(v_sb[:, i:i + 1], v_ps[:, i:i + 1])

        v_bounce = dram.tile([128 * pm_cols], F32, name="v_bounce")
        v_red = dram.tile([128 * pm_cols], F32, name="v_red")
        nc.sync.dma_start(v_bounce[:].rearrange("(p t) -> p t", p=128), v_sb[:])
        nc.gpsimd.collective_compute(
            "AllReduce", ALU.add, replica_groups=groups,
            ins=[v_bounce.opt()], outs=[v_red.opt()],
        )
        v_rd = sb.tile([128, pm_cols], F32, name="v_rd")
        nc.sync.dma_start(v_rd[:], v_red[:].rearrange("(p t) -> p t", p=128))
        v_pm = sb.tile([128, pm_cols], BF16, name="v_pm")
        nc.scalar.copy(v_pm[:], v_rd[:])

        # ---- main pass over s-blocks: score -> exp -> context partials ----
        e_rows = sb.tile([1, s_shard], F32, name="e_rows")
        mb_row = sb.tile([1, nb], F32, name="mb_row")
        negmb_row = sb.tile([1, nb], F32, name="negmb_row")
        zb_row = sb.tile([1, nb], F32, name="zb_row")
        ctx_store = sb.tile([128, ht_tiles * nb], F32, name="ctx_store")

        for b in range(nb):
            hid_sb = hidp.tile([128, ht_tiles * blk], BF16, name="hid_sb",
                               tag="hid")
            for g in range(sub):
                tpg = ht_tiles // sub  # h-tiles per sub-DMA
                nc.sync.dma_start(
                    hid_sb[:, g * tpg * blk:(g + 1) * tpg * blk]
                    .rearrange("p (t s) -> p t s", t=tpg),
                    hid_t.ap()[g * tpg * 128:(g + 1) * tpg * 128,
                               b * blk:(b + 1) * blk]
                    .rearrange("(t p) s -> p t s", p=128),
                )

            score_ps = psum2.tile([1, blk], F32, name="score_ps", tag="score")
            for t in range(ht_tiles):
                nc.tensor.matmul(
                    score_ps[:],
                    lhsT=v_pm[:, t:t + 1],
                    rhs=hid_sb[:, t * blk:(t + 1) * blk],
                    start=(t == 0), stop=(t == ht_tiles - 1),
                )

            nc.vector.reduce_max(out=mb_row[:, b:b + 1], in_=score_ps[:],
                                 axis=mybir.AxisListType.X)
            nc.scalar.mul(negmb_row[:, b:b + 1], mb_row[:, b:b + 1], -1.0)
            # e = exp(score - m_b); Z_b accumulated by the activation engine
            nc.scalar.activation(
                e_rows[:, b * blk:(b + 1) * blk], score_ps[:], AF.Exp,
                bias=negmb_row[:, b:b + 1], scale=1.0,
                accum_out=zb_row[:, b:b + 1],
            )
            # broadcast e across partitions via PE, cast to bf16
            e_bc_ps = psum2.tile([128, blk], F32, name="e_bc_ps", tag="ebc")
            nc.tensor.matmul(e_bc_ps[:], lhsT=ones[:],
                             rhs=e_rows[:, b * blk:(b + 1) * blk],
                             start=True, stop=True)
            e_bc = sb2.tile([128, blk], BF16, name="e_bc", tag="ebc_sb")
            nc.scalar.copy(e_bc[:], e_bc_ps[:])
            # context partials: ctx_store[:, b*HT + t] = sum_s hid*e
            # (tensor_tensor_reduce is HW-fatal on this runtime; use two ops)
            for t in range(ht_tiles):
                tt_out = sb2.tile([128, blk], BF16, name="tt_out", tag="tt")
                nc.vector.tensor_mul(tt_out[:],
                                     hid_sb[:, t * blk:(t + 1) * blk], e_bc[:])
                nc.vector.reduce_sum(
                    out=ctx_store[:, b * ht_tiles + t:b * ht_tiles + t + 1],
                    in_=tt_out[:], axis=mybir.AxisListType.X)

        # ---- local stats -> AllGather -> global softmax factors ----
        neg_m_core = sb.tile([1, 1], F32, name="neg_m_core")
        nc.vector.tensor_reduce(out=neg_m_core[:], in_=mb_row[:],
                                axis=mybir.AxisListType.X, op=ALU.max,
                                negate=True)
        alpha = sb.tile([1, nb], F32, name="alpha")
        nc.scalar.activation(alpha[:], negmb_row[:], AF.Exp,
                             bias=neg_m_core[:], scale=-1.0)
        scr_nb = sb.tile([1, nb], F32, name="scr_nb")
        z_core = sb.tile([1, 1], F32, name="z_core")
        nc.vector.tensor_mul(scr_nb[:], alpha[:], zb_row[:])
        nc.vector.reduce_sum(out=z_core[:], in_=scr_nb[:],
                             axis=mybir.AxisListType.X)
        stats_sb = sb.tile([1, 2], F32, name="stats_sb")
        nc.scalar.mul(stats_sb[:, 0:1], neg_m_core[:], -1.0)
        nc.scalar.copy(stats_sb[:, 1:2], z_core[:])

        stats_bounce = dram.tile([2], F32, name="stats_bounce")
        stats_all = dram.tile([2 * n_cores], F32, name="stats_all")
        nc.sync.dma_start(stats_bounce[:].rearrange("(p f) -> p f", p=1),
                          stats_sb[:])
        nc.gpsimd.collective_compute(
            "AllGather", ALU.bypass, replica_groups=groups,
            ins=[stats_bounce.opt()], outs=[stats_all.opt()],
        )
        m_all = sb.tile([1, n_cores], F32, name="m_all")
        z_all = sb.tile([1, n_cores], F32, name="z_all")
        strided = stats_all[:].rearrange("(r two) -> two r", two=2)
        nc.sync.dma_start(m_all[:], strided[0:1, :])
        nc.sync.dma_start(z_all[:], strided[1:2, :])

        neg_m_g = sb.tile([1, 1], F32, name="neg_m_g")
        nc.vector.tensor_reduce(out=neg_m_g[:], in_=m_all[:],
                                axis=mybir.AxisListType.X, op=ALU.max,
                                negate=True)
        beta = sb.tile([1, n_cores], F32, name="beta")
        nc.scalar.activation(beta[:], m_all[:], AF.Exp, bias=neg_m_g[:],
                             scale=1.0)
        scr_nc = sb.tile([1, n_cores], F32, name="scr_nc")
        z_g = sb.tile([1, 1], F32, name="z_g")
        nc.vector.tensor_mul(scr_nc[:], beta[:], z_all[:])
        nc.vector.reduce_sum(out=z_g[:], in_=scr_nc[:],
                             axis=mybir.AxisListType.X)
        inv_zg = sb.tile([1, 1], F32, name="inv_zg")
        nc.vector.reciprocal(inv_zg[:], z_g[:])
        # gamma_b = exp(m_b - m_g) / Z_g
        gamma = sb.tile([1, nb], F32, name="gamma")
        nc.scalar.activation(gamma[:], negmb_row[:], AF.Exp, bias=neg_m_g[:],
                             scale=-1.0)
        gamma2 = sb.tile([1, nb], F32, name="gamma2")
        nc.vector.tensor_scalar_mul(gamma2[:], gamma[:], inv_zg[:])

        # ---- attention weights output ----
        w_row = sb.tile([1, s_shard], F32, name="w_row")
        for b in range(nb):
            nc.vector.tensor_scalar_mul(w_row[:, b * blk:(b + 1) * blk],
                                        e_rows[:, b * blk:(b + 1) * blk],
                                        gamma2[:, b:b + 1])
        nc.sync.dma_start(out_w.ap().rearrange("(p f) -> p f", p=1), w_row[:])

        # ---- combine context partials, AllReduce ----
        gam_ps = psum.tile([128, nb], F32, name="gam_ps")
        nc.tensor.matmul(gam_ps[:], lhsT=ones[:], rhs=gamma2[:],
                         start=True, stop=True)
        gam_sb = sb.tile([128, nb], F32, name="gam_sb")
        nc.scalar.copy(gam_sb[:], gam_ps[:])

        ctx_acc = sb.tile([128, ht_tiles], F32, name="ctx_acc")
        ctx_tmp = sb.tile([128, ht_tiles], F32, name="ctx_tmp")
        nc.vector.tensor_scalar_mul(
            ctx_acc[:], ctx_store[:, 0:ht_tiles], gam_sb[:, 0:1])
        for b in range(1, nb):
            nc.vector.tensor_scalar_mul(
                ctx_tmp[:], ctx_store[:, b * ht_tiles:(b + 1) * ht_tiles],
                gam_sb[:, b:b + 1])
            nc.vector.tensor_add(ctx_acc[:], ctx_acc[:], ctx_tmp[:])

        ctx_bounce = dram.tile([h], F32, name="ctx_bounce")
        ctx_red = dram.tile([h], F32, name="ctx_red")
        nc.sync.dma_start(ctx_bounce[:].rearrange("(p t) -> p t", p=128),
                          ctx_acc[:])
        nc.gpsimd.collective_compute(
            "AllReduce", ALU.add, replica_groups=groups,
            ins=[ctx_bounce.opt()], outs=[ctx_red.opt()],
        )
        ctx_rd = sb.tile([128, ht_tiles], F32, name="ctx_rd")
        nc.sync.dma_start(ctx_rd[:], ctx_red[:].rearrange("(p t) -> p t", p=128))

        # ---- fc2: av = tanh(W2 @ concat(ctx, h_t)), this core's 512 rows ----
        pre_pm = sb.tile([128, pm_cols + ht_tiles], BF16, name="pre_pm")
        nc.scalar.copy(pre_pm[:, 0:ht_tiles], ctx_rd[:])
        nc.sync.dma_start(pre_pm[:, ht_tiles:ht_tiles + pm_cols], ht_pm.ap())

        w2_sb = sb.tile([128, k2 * m_shard], BF16, name="w2_sb")
        w2_sub = 8
        tpg2 = k2 // w2_sub
        for g in range(w2_sub):
            nc.sync.dma_start(
                w2_sb[:, g * tpg2 * m_shard:(g + 1) * tpg2 * m_shard]
                .rearrange("p (t m) -> p t m", t=tpg2),
                w2t.ap()[g * tpg2 * 128:(g + 1) * tpg2 * 128, :]
                .rearrange("(t p) m -> p t m", p=128),
            )
        fc2_ps = psum.tile([1, m_shard], F32, name="fc2_ps")
        for k in range(k2):
            nc.tensor.matmul(
                fc2_ps[:],
                lhsT=pre_pm[:, k:k + 1],
                rhs=w2_sb[:, k * m_shard:(k + 1) * m_shard],
                start=(k == 0), stop=(k == k2 - 1),
            )
        av_row = sb.tile([1, m_shard], F32, name="av_row")
        nc.scalar.activation(av_row[:], fc2_ps[:], AF.Tanh)
        nc.sync.dma_start(out_av.ap().rearrange("(p f) -> p f", p=1), av_row[:])

    nc.compile()
    return nc


def shard_inputs(hidden_states, W1, W2, n_cores=N_CORES):
    bf = ml_dtypes.bfloat16
    s, h = hidden_states.shape
    s_sh = s // n_cores
    m_sh = h // n_cores
    jt = (h // n_cores) // 128
    ht = np.ascontiguousarray(hidden_states[-1])          # [h] f32
    ht_pm = np.ascontiguousarray(ht.reshape(h // 128, 128).T).astype(bf)
    in_maps = []
    for c in range(n_cores):
        rows = hidden_states[c * s_sh:(c + 1) * s_sh]
        hid_t = np.ascontiguousarray(rows.T).astype(bf)   # [h, s_sh]
        w1s = W1[c * m_sh:(c + 1) * m_sh, :].astype(bf)
        w2t = np.ascontiguousarray(W2[c * m_sh:(c + 1) * m_sh, :].T).astype(bf)
        ht_loc = np.ascontiguousarray(ht_pm[:, c * jt:(c + 1) * jt])
        in_maps.append({
            "hid_t": hid_t, "w1s": w1s, "w2t": w2t,
            "ht_pm": ht_pm, "ht_loc": ht_loc,
        })
    return in_maps


_GRAPH = None
TRACE = False          # set True (e.g. from test.py) to capture an NTFF profile
TMPDIR = None          # optional trace output dir
LAST_RESULTS = None    # BassKernelResults of the most recent run


def kernel(hidden_states, W1, W2):
    global _GRAPH, LAST_RESULTS
    hidden_states = np.asarray(hidden_states, dtype=np.float32)
    W1 = np.asarray(W1, dtype=np.float32)
    W2 = np.asarray(W2, dtype=np.float32)
    if _GRAPH is None:
        _GRAPH = build_graph()
    in_maps = shard_inputs(hidden_states, W1, W2)
    res = run_bass_kernel_spmd(_GRAPH, in_maps, core_ids=list(range(N_CORES)),
                               trace=TRACE, tmpdir=TMPDIR)
    LAST_RESULTS = res
    outs = res.results
    aw = np.concatenate([outs[c]["out_w"] for c in range(N_CORES)])
    av = np.concatenate([outs[c]["out_av"] for c in range(N_CORES)])
    return av.astype(np.float32), aw.astype(np.float32)
